# revision 5
# baseline (speedup 1.0000x reference)
"""Trainium2 Bass kernel for per-sample modulated/demodulated 3x3 conv.

Problem: x (8,512,32,32), s (8,512), w (512,512,3,3) ->
  wm[b,o,i,ky,kx] = w * (s[b,i]+1); demod by rsqrt(sum wm^2 + eps) per (b,o);
  y[b] = conv2d_same(x[b], wm[b]).

Winograd F(2x2,3x3) formulation: y = A^T [ (G w G^T) . (B^T x' B) ] A with
modulation folded into x (x' = x*(1+s)) and demodulation folded into the
output (y *= den[o], den = rsqrt(sum_i (1+s_i)^2 wsq[i,o] + eps)). The
weight transform G w G^T and wsq = sum_pos w^2 are weight-only
preprocessing done host-side (like the w9 repacking the direct kernel
did); weights ship as bf16.

Sharding: 4 batch-groups (2 samples each) x 2 cout-halves = 8 cores. Each
core loads half the Winograd weights (4 MiB bf16), both its samples'
pre-padded bf16 x, and computes y for (2 samples x 256 couts). The input
transform V = B^T x' B is computed once per sample and reused by both
cout blocks; the two samples are concatenated along matmul columns so
every PE matmul streams the full 512-column moving max.

Per-core dataflow:
  - ACT: x modulation (bf16 in/out, per-partition (1+s) scale), demod
    sqrt, final scaled strided scatter-stores into y tiles.
  - DVE: input transform (two butterfly passes over stride-2 rearrange
    views, all bf16), half of output-transform step A (PSUM reads), all
    of step B, reciprocal.
  - GPSIMD: x/s DMA triggers (SWDGE), other half of step A.
  - PE: 16 tiny demod matvecs early, then 8 stages x 16 matmuls of 512
    cols (bf16) accumulating over cin chunks; 4 PSUM banks per stage,
    banks recycled two stages later after step A drains them.
  - Output transform: stage (ob,pw) holds M[ph] in 4 banks; step A
    contracts ph -> sA (bf16), step B contracts pw -> z (bf16), ACT
    scatters z*den into y (stride-2 interleave), y ships bf16.
"""

import sys

if "/opt/trn_rl_repo" not in sys.path:
    sys.path.insert(0, "/opt/trn_rl_repo")

import numpy as np
import ml_dtypes

B = 8
CIN = 512
COUT = 512
H = 32
W = 32
NCH = CIN // 128   # cin chunks
PGRID = 4          # batch groups
QGRID = 2          # cout halves
SPC = B // PGRID   # samples per core
OHALF = COUT // QGRID
OB = OHALF // 128  # cout blocks per core
HP = H + 2
EPS = 1e-8
BF = ml_dtypes.bfloat16

_compiled_nc = None


def _build():
    import concourse.tile as tile
    from concourse import bacc, mybir

    F32 = mybir.dt.float32
    BF16 = mybir.dt.bfloat16

    nc = bacc.Bacc("TRN2", target_bir_lowering=False, debug=False, num_devices=B)
    xp_d = nc.dram_tensor("xp", [SPC, NCH, 128, HP, HP], BF16, kind="ExternalInput").ap()
    s_d = nc.dram_tensor("sp", [128, NCH, SPC], F32, kind="ExternalInput").ap()
    wt_d = nc.dram_tensor("wt", [NCH, 128, 4, 4, OHALF], BF16, kind="ExternalInput").ap()
    wsq_d = nc.dram_tensor("wsq", [128, NCH, OHALF], BF16, kind="ExternalInput").ap()
    y_d = nc.dram_tensor("y", [SPC, OB, 128, H * W], BF16, kind="ExternalOutput").ap()

    with tile.TileContext(nc) as tc:
        with (
            tc.tile_pool(name="wpool", bufs=1) as wpool,
            tc.tile_pool(name="xstage", bufs=4) as xstage,
            tc.tile_pool(name="xmp", bufs=1) as xmp,
            tc.tile_pool(name="tp", bufs=1) as tp,
            tc.tile_pool(name="vp", bufs=1) as vp,
            tc.tile_pool(name="sap", bufs=1) as sap,
            tc.tile_pool(name="up", bufs=4) as up,
            tc.tile_pool(name="zp", bufs=4) as zp,
            tc.tile_pool(name="yp", bufs=1) as yp,
            tc.tile_pool(name="misc", bufs=1) as misc,
            tc.tile_pool(name="psum", bufs=8, space="PSUM") as psum,
        ):
            wt_sb = [
                wpool.tile([128, 4, 4, OHALF], BF16, name=f"wt{c}", tag=f"wt{c}")
                for c in range(NCH)
            ]
            xm = [
                [xmp.tile([128, HP, HP], BF16, name=f"xm{sm}_{c}", tag=f"xm{sm}_{c}")
                 for c in range(NCH)]
                for sm in range(SPC)
            ]
            T = [
                [[tp.tile([128, 16, HP], BF16, name=f"t{sm}_{c}_{a}", tag=f"t{sm}_{c}_{a}")
                  for a in range(4)]
                 for c in range(NCH)]
                for sm in range(SPC)
            ]
            V = [
                [vp.tile([128, SPC, 16, 16], BF16, name=f"v{c}_{p}", tag=f"v{c}_{p}")
                 for p in range(16)]
                for c in range(NCH)
            ]
            sA = [
                [[sap.tile([128, 512], BF16, name=f"sa{ob}_{j}_{b}", tag=f"sa{ob}_{j}_{b}")
                  for b in range(4)]
                 for j in range(2)]
                for ob in range(OB)
            ]
            y_sb = [
                [yp.tile([128, H, W], BF16, name=f"y{sm}_{ob}", tag=f"y{sm}_{ob}")
                 for ob in range(OB)]
                for sm in range(SPC)
            ]
            s_sb = misc.tile([128, NCH, SPC], F32, name="s_sb", tag="s_sb")
            s1 = misc.tile([128, NCH, SPC], F32, name="s1", tag="s1")
            q = misc.tile([128, NCH, SPC], BF16, name="q", tag="q")
            wsq_sb = misc.tile([128, NCH, OHALF], BF16, name="wsq_sb", tag="wsq_sb")
            den_s = misc.tile([128, OB, SPC], F32, name="den_s", tag="den_s")
            den = misc.tile([128, OB, SPC], F32, name="den", tag="den")
            eps_t = misc.tile([128, 1], F32, name="eps_t", tag="eps_t")
            junk = misc.tile([128, 512], BF16, name="junk", tag="junk")

            nc.vector.memset(eps_t, EPS)
            nc.vector.memset(junk, 0.0)

            # --- PE warmup while the first DMAs land (HAM clock gate)
            warm = psum.tile([128, 512], F32, name="warm", tag="acc")
            for _ in range(8):
                nc.tensor.matmul(warm, lhsT=junk[:, 0:128], rhs=junk,
                                 start=True, stop=True)

            # --- DMAs: s + x on the gpsimd SWDGE queue, wt + wsq on sync.
            nc.gpsimd.dma_start(out=s_sb, in_=s_d)
            nc.sync.dma_start(out=wsq_sb, in_=wsq_d)
            # c0 weights per pw-group so stage (ob0,pw0) is unblocked first
            for b in range(4):
                nc.sync.dma_start(out=wt_sb[0][:, b, :, :], in_=wt_d[0, :, b, :, :])
            xs = {}
            for c in range(NCH):
                for sm in range(SPC):
                    t = xstage.tile([128, HP, HP], BF16, name=f"xs{sm}_{c}",
                                    tag="xstage", bufs=4)
                    xs[(sm, c)] = t
                    nc.gpsimd.dma_start(out=t, in_=xp_d[sm, c])
            for c in range(1, NCH):
                nc.sync.dma_start(out=wt_sb[c], in_=wt_d[c])

            # --- demod: q = (1+s)^2 ; dsum[ob] = sum_c wsq[c,ob-block]^T q
            nc.vector.tensor_scalar_add(s1, s_sb, 1.0)
            nc.vector.tensor_mul(q, s1, s1)
            dsum = [psum.tile([128, SPC], F32, name=f"dsum{ob}", tag="acc")
                    for ob in range(OB)]
            for ob in range(OB):
                for c in range(NCH):
                    nc.tensor.matmul(
                        dsum[ob],
                        lhsT=wsq_sb[:, c, ob * 128:(ob + 1) * 128],
                        rhs=q[:, c, :],
                        start=(c == 0), stop=(c == NCH - 1),
                    )

            # --- modulation (ACT) + demod sqrt interleaved
            for c in range(NCH):
                for sm in range(SPC):
                    nc.scalar.mul(xm[sm][c], xs[(sm, c)], s1[:, c, sm:sm + 1])
                if c == 0:
                    for ob in range(OB):
                        nc.scalar.activation(
                            den_s[:, ob, :], dsum[ob],
                            mybir.ActivationFunctionType.Sqrt, bias=eps_t)

            # --- input transform step 1 (rows): T[a] from stride-2 row views
            for c in range(NCH):
                for sm in range(SPC):
                    xr = xm[sm][c].rearrange("p (a b) w -> p a b w", b=2)
                    ev0, ev1 = xr[:, 0:16, 0, :], xr[:, 1:17, 0, :]
                    od0, od1 = xr[:, 0:16, 1, :], xr[:, 1:17, 1, :]
                    Tc = T[sm][c]
                    nc.vector.tensor_sub(Tc[0], ev0, ev1)
                    nc.vector.tensor_add(Tc[1], od0, ev1)
                    nc.vector.tensor_sub(Tc[2], ev1, od0)
                    nc.vector.tensor_sub(Tc[3], od0, od1)
                if c == 0:
                    nc.vector.reciprocal(den, den_s)

            # --- input transform step 2 (cols), emitted in pw-blocks so the
            # first PE stage (pw=0) is unblocked after one block
            def step2_block(b):
                for c in range(NCH):
                    for a in range(4):
                        for sm in range(SPC):
                            Tr = T[sm][c][a].rearrange("p t (a b) -> p t a b", b=2)
                            ev0, ev1 = Tr[:, :, 0:16, 0], Tr[:, :, 1:17, 0]
                            od0, od1 = Tr[:, :, 0:16, 1], Tr[:, :, 1:17, 1]
                            out = V[c][a * 4 + b][:, sm]
                            if b == 0:
                                nc.vector.tensor_sub(out, ev0, ev1)
                            elif b == 1:
                                nc.vector.tensor_add(out, od0, ev1)
                            elif b == 2:
                                nc.vector.tensor_sub(out, ev1, od0)
                            else:
                                nc.vector.tensor_sub(out, od0, od1)

            for b in range(3):
                step2_block(b)

            # --- conv stages: (ob, pw) x (cin chunk, ph) matmuls of 512 cols
            M = {}

            def stage(ob, b):
                for c in range(NCH):
                    for a in range(4):
                        key = (ob, b, a)
                        if key not in M:
                            M[key] = psum.tile([128, 512], F32,
                                               name=f"m{ob}_{b}_{a}", tag="acc")
                        nc.tensor.matmul(
                            M[key],
                            lhsT=wt_sb[c][:, b, a, ob * 128:(ob + 1) * 128],
                            rhs=V[c][a * 4 + b].rearrange("p s t u -> p (s t u)"),
                            start=(c == 0), stop=(c == NCH - 1),
                        )

            def step_a(ob, b):
                # sA[0] = M0+M1+M2, sA[1] = M1-M2-M3. TensorTensor may read
                # only ONE operand from PSUM, so ACT first drains M1/M2 to
                # SBUF; every TT op below touches at most one PSUM bank.
                m = [M[(ob, b, a)] for a in range(4)]
                d1 = up.tile([128, 512], F32, name=f"d1_{ob}_{b}", tag="u", bufs=8)
                d2 = up.tile([128, 512], F32, name=f"d2_{ob}_{b}", tag="u", bufs=8)
                nc.scalar.copy(d1, m[1])
                nc.scalar.copy(d2, m[2])
                u0 = up.tile([128, 512], F32, name=f"u0_{ob}_{b}", tag="u", bufs=8)
                nc.vector.tensor_add(u0, m[0], d1)
                nc.vector.tensor_add(sA[ob][0][b], u0, d2)
                u1 = up.tile([128, 512], F32, name=f"u1_{ob}_{b}", tag="u", bufs=8)
                nc.gpsimd.tensor_sub(u1, d1, d2)
                nc.vector.tensor_sub(sA[ob][1][b], u1, m[3])

            def step_b_and_store(ob):
                for j in range(2):  # j = ih (output row parity)
                    sj = sA[ob][j]
                    v0 = up.tile([128, 512], F32, name=f"v0_{ob}_{j}", tag="u", bufs=8)
                    z0 = zp.tile([128, SPC, 16, 16], BF16, name=f"z{ob}_{j}0",
                                 tag="z", bufs=4)
                    zf0 = z0.rearrange("p s t u -> p (s t u)")
                    nc.vector.tensor_add(v0, sj[0], sj[1])
                    nc.vector.tensor_add(zf0, v0, sj[2])
                    v1 = up.tile([128, 512], F32, name=f"v1_{ob}_{j}", tag="u", bufs=8)
                    z1 = zp.tile([128, SPC, 16, 16], BF16, name=f"z{ob}_{j}1",
                                 tag="z", bufs=4)
                    zf1 = z1.rearrange("p s t u -> p (s t u)")
                    nc.vector.tensor_sub(v1, sj[1], sj[2])
                    nc.vector.tensor_sub(zf1, v1, sj[3])
                    for iw, z in ((0, z0), (1, z1)):
                        for sm in range(SPC):
                            yv = y_sb[sm][ob].rearrange(
                                "p (t i) (u j) -> p t i u j", i=2, j=2)
                            nc.scalar.mul(
                                yv[:, :, j, :, iw], z[:, sm],
                                den[:, ob, sm:sm + 1])
                for sm in range(SPC):
                    nc.scalar.dma_start(
                        out=y_d[sm, ob],
                        in_=y_sb[sm][ob].rearrange("p a b -> p (a b)"))

            first = True
            for ob in range(OB):
                for b in range(4):
                    stage(ob, b)
                    step_a(ob, b)
                    if first:
                        step2_block(3)
                        first = False
                step_b_and_store(ob)

    nc.compile()
    return nc


_G = np.array(
    [[1.0, 0.0, 0.0], [0.5, 0.5, 0.5], [0.5, -0.5, 0.5], [0.0, 0.0, 1.0]],
    np.float32)


def prepare_in_maps(x, s, w):
    """Shard + pack full inputs into per-core in_maps (core = g*QGRID + h)."""
    x = np.asarray(x, dtype=np.float32)
    s = np.asarray(s, dtype=np.float32)
    w = np.asarray(w, dtype=np.float32)

    # Winograd weights: wt4[a,b,i,o] = sum_pq G[a,p] G[b,q] w[o,i,p,q]
    wt4 = np.einsum("ap,bq,oipq->abio", _G, _G, w, optimize=True)
    # -> [cin, b, a, cout] -> [NCH,128,4,4,COUT]
    wt_l = np.ascontiguousarray(wt4.transpose(2, 1, 0, 3)).reshape(
        NCH, 128, 4, 4, COUT).astype(BF)
    wsq = np.sum(w * w, axis=(2, 3)).T  # [cin, cout]
    wsq_l = np.ascontiguousarray(
        wsq.reshape(NCH, 128, COUT).transpose(1, 0, 2)).astype(BF)

    x_bf = x.astype(BF).reshape(PGRID, SPC, NCH, 128, H, W)
    xp_all = np.zeros((PGRID, SPC, NCH, 128, HP, HP), BF)
    xp_all[:, :, :, :, 1:H + 1, 1:W + 1] = x_bf
    s_all = s.reshape(PGRID, SPC, NCH, 128).transpose(0, 3, 2, 1)

    in_maps = []
    for g in range(PGRID):
        for h in range(QGRID):
            in_maps.append({
                "xp": np.ascontiguousarray(xp_all[g]),
                "sp": np.ascontiguousarray(s_all[g]),
                "wt": np.ascontiguousarray(wt_l[:, :, :, :, h * OHALF:(h + 1) * OHALF]),
                "wsq": np.ascontiguousarray(wsq_l[:, :, h * OHALF:(h + 1) * OHALF]),
            })
    return in_maps


def assemble_output(results):
    y = np.zeros((B, COUT, H, W), np.float32)
    for g in range(PGRID):
        for h in range(QGRID):
            r = results[g * QGRID + h]["y"].astype(np.float32)
            for sm in range(SPC):
                for ob in range(OB):
                    y[g * SPC + sm,
                      h * OHALF + ob * 128:h * OHALF + (ob + 1) * 128] = (
                        r[sm, ob].reshape(128, H, W))
    return y


def kernel(x, s, w):
    from concourse.bass_utils import run_bass_kernel_spmd

    global _compiled_nc
    if _compiled_nc is None:
        _compiled_nc = _build()
    nc = _compiled_nc

    in_maps = prepare_in_maps(x, s, w)
    res = run_bass_kernel_spmd(nc, in_maps, list(range(B))).results
    return assemble_output(res)


# revision 6
# speedup vs baseline: 1.6305x; 1.6305x over previous
"""Trainium2 Bass kernel for per-sample modulated/demodulated 3x3 conv.

Problem: x (8,512,32,32), s (8,512), w (512,512,3,3) ->
  wm[b,o,i,ky,kx] = w * (s[b,i]+1); demod by rsqrt(sum wm^2 + eps) per (b,o);
  y[b] = conv2d_same(x[b], wm[b]).

1D Winograd F(2,3) along H: y = A^T [ (G_h w) . (B_h^T x') ] with the kx
taps handled directly via shifted-window PSUM accumulation (like a plain
conv), modulation folded into x (x' = x*(1+s)) and demodulation folded
into the output (y *= den[o], den = rsqrt(sum_i (1+s_i)^2 wsq[i,o]+eps)).
The h-transform of the weights (G w along ky -> 4 positions) and wsq are
weight-only preprocessing done host-side; weights ship bf16. PE work is
2/3 of a direct conv; the input transform is 4 contiguous-stride DVE ops
per (cin chunk, sample) (stride-2 only on the middle dim, which DVE runs
at full rate -- inner-stride-2 patterns measured ~2x slower).

Sharding: 4 batch-groups (2 samples each) x 2 cout-halves = 8 cores; each
core loads half the transformed weights (3 MiB bf16) and its 2 samples.

Per-core schedule: two PE waves (sample 0, then sample 1), 8 PSUM banks
per wave (2 cout blocks x 4 h-positions), each bank accumulating 12
matmuls (4 cin chunks x 3 kx shifts) of ~512 bf16 columns. DVE makes
V = B^T x' per chunk (sample 0 first) so wave A starts after one chunk's
4 transform ops. Output transform A^T (z0 = M0+M1+M2 even rows,
z1 = M1-M2-M3 odd rows) splits PSUM reads one-per-op (HW limit): ACT
drains M1/M2 to SBUF, DVE does the adds, ACT applies den and scatters
into even/odd row pairs of the bf16 y tile.
"""

import sys

if "/opt/trn_rl_repo" not in sys.path:
    sys.path.insert(0, "/opt/trn_rl_repo")

import numpy as np
import ml_dtypes

B = 8
CIN = 512
COUT = 512
H = 32
W = 32
NCH = CIN // 128   # cin chunks
PGRID = 4          # batch groups
QGRID = 2          # cout halves
SPC = B // PGRID   # samples per core
OHALF = COUT // QGRID
OB = OHALF // 128  # cout blocks per core
HP = H + 2
EPS = 1e-8
BF = ml_dtypes.bfloat16

_compiled_nc = None


def _build():
    import concourse.tile as tile
    from concourse import bacc, mybir

    F32 = mybir.dt.float32
    BF16 = mybir.dt.bfloat16

    nc = bacc.Bacc("TRN2", target_bir_lowering=False, debug=False, num_devices=B)
    xp_d = nc.dram_tensor("xp", [SPC, NCH, 128, HP, HP], BF16, kind="ExternalInput").ap()
    s_d = nc.dram_tensor("sp", [128, NCH, SPC], F32, kind="ExternalInput").ap()
    wt_d = nc.dram_tensor("wt", [NCH, 128, 4, 3, OHALF], BF16, kind="ExternalInput").ap()
    wsq_d = nc.dram_tensor("wsq", [128, NCH, OHALF], BF16, kind="ExternalInput").ap()
    y_d = nc.dram_tensor("y", [SPC, OB, 128, H * W], BF16, kind="ExternalOutput").ap()

    with tile.TileContext(nc) as tc:
        with (
            tc.tile_pool(name="wpool", bufs=1) as wpool,
            tc.tile_pool(name="xstage", bufs=4) as xstage,
            tc.tile_pool(name="xmp", bufs=1) as xmp,
            tc.tile_pool(name="vp", bufs=1) as vp,
            tc.tile_pool(name="up", bufs=6) as up,
            tc.tile_pool(name="zp", bufs=4) as zp,
            tc.tile_pool(name="yp", bufs=1) as yp,
            tc.tile_pool(name="misc", bufs=1) as misc,
            tc.tile_pool(name="psum", bufs=8, space="PSUM") as psum,
        ):
            wt_sb = [
                wpool.tile([128, 4, 3, OHALF], BF16, name=f"wt{c}", tag=f"wt{c}")
                for c in range(NCH)
            ]
            xm = [
                [xmp.tile([128, HP, HP], BF16, name=f"xm{sm}_{c}", tag=f"xm{sm}_{c}")
                 for c in range(NCH)]
                for sm in range(SPC)
            ]
            # V[sm][c][a]: h-transformed input, [128, 16 th, 34 w(padded)]
            V = [
                [[vp.tile([128, 16, HP], BF16, name=f"v{sm}_{c}_{a}", tag=f"v{sm}_{c}_{a}")
                  for a in range(4)]
                 for c in range(NCH)]
                for sm in range(SPC)
            ]
            y_sb = [
                [yp.tile([128, H, W], BF16, name=f"y{sm}_{ob}", tag=f"y{sm}_{ob}")
                 for ob in range(OB)]
                for sm in range(SPC)
            ]
            s_sb = misc.tile([128, NCH, SPC], F32, name="s_sb", tag="s_sb")
            s1 = misc.tile([128, NCH, SPC], F32, name="s1", tag="s1")
            q = misc.tile([128, NCH, SPC], BF16, name="q", tag="q")
            wsq_sb = misc.tile([128, NCH, OHALF], BF16, name="wsq_sb", tag="wsq_sb")
            den_s = misc.tile([128, OB, SPC], F32, name="den_s", tag="den_s")
            den = misc.tile([128, OB, SPC], F32, name="den", tag="den")
            eps_t = misc.tile([128, 1], F32, name="eps_t", tag="eps_t")
            junk = misc.tile([128, 512], BF16, name="junk", tag="junk")

            nc.vector.memset(eps_t, EPS)
            nc.vector.memset(junk, 0.0)

            # --- PE warmup while the first DMAs land (HAM clock gate)
            warm = psum.tile([128, 512], F32, name="warm", tag="acc")
            for _ in range(8):
                nc.tensor.matmul(warm, lhsT=junk[:, 0:128], rhs=junk,
                                 start=True, stop=True)

            # --- DMAs: s + x on the gpsimd SWDGE queue, wt + wsq on sync.
            nc.gpsimd.dma_start(out=s_sb, in_=s_d)
            nc.sync.dma_start(out=wsq_sb, in_=wsq_d)
            # c0 weights per h-position so wave A's first matmuls unblock early
            for a in range(4):
                nc.sync.dma_start(out=wt_sb[0][:, a, :, :], in_=wt_d[0, :, a, :, :])
            xs = {}
            for sm in range(SPC):          # sample 0 chunks first
                for c in range(NCH):
                    t = xstage.tile([128, HP, HP], BF16, name=f"xs{sm}_{c}",
                                    tag="xstage", bufs=4)
                    xs[(sm, c)] = t
                    nc.gpsimd.dma_start(out=t, in_=xp_d[sm, c])
            for c in range(1, NCH):
                nc.sync.dma_start(out=wt_sb[c], in_=wt_d[c])

            # --- demod: q = (1+s)^2 ; dsum[ob] = sum_c wsq[c,ob-block]^T q
            nc.vector.tensor_scalar_add(s1, s_sb, 1.0)
            nc.vector.tensor_mul(q, s1, s1)
            dsum = [psum.tile([128, SPC], F32, name=f"dsum{ob}", tag="acc")
                    for ob in range(OB)]
            for ob in range(OB):
                for c in range(NCH):
                    nc.tensor.matmul(
                        dsum[ob],
                        lhsT=wsq_sb[:, c, ob * 128:(ob + 1) * 128],
                        rhs=q[:, c, :],
                        start=(c == 0), stop=(c == NCH - 1),
                    )

            # --- modulation (ACT, sample 0 first) + demod sqrt interleaved
            for sm in range(SPC):
                for c in range(NCH):
                    nc.scalar.mul(xm[sm][c], xs[(sm, c)], s1[:, c, sm:sm + 1])
                    if sm == 0 and c == 0:
                        for ob in range(OB):
                            nc.scalar.activation(
                                den_s[:, ob, :], dsum[ob],
                                mybir.ActivationFunctionType.Sqrt, bias=eps_t)

            # --- input transform (DVE): V[a] over stride-2 row views of xm
            for sm in range(SPC):
                for c in range(NCH):
                    xr = xm[sm][c].rearrange("p (a b) w -> p a b w", b=2)
                    ev0, ev1 = xr[:, 0:16, 0, :], xr[:, 1:17, 0, :]
                    od0, od1 = xr[:, 0:16, 1, :], xr[:, 1:17, 1, :]
                    Vc = V[sm][c]
                    nc.vector.tensor_sub(Vc[0], ev0, ev1)
                    nc.vector.tensor_add(Vc[1], od0, ev1)
                    nc.vector.tensor_sub(Vc[2], ev1, od0)
                    nc.vector.tensor_sub(Vc[3], od0, od1)
                    if sm == 0 and c == 0:
                        nc.vector.reciprocal(den, den_s)

            # --- conv waves: per sample, 8 banks = (2 ob x 4 a), each
            # accumulating 12 matmuls (4 c x 3 kx shifted windows). The
            # zero-pad cols of V contribute nothing, so kx=0 skips out col 0
            # and kx=2 skips out col 31 (PSUM has_written covers first hits).
            M = {}

            def wave(sm):
                for c in range(NCH):
                    # last chunk: ob0 first so its drain overlaps ob1 matmuls
                    for ob in range(OB):
                        for a in range(4):
                            key = (sm, ob, a)
                            if key not in M:
                                M[key] = psum.tile([128, 16, W], F32,
                                                   name=f"m{sm}_{ob}_{a}", tag="acc")
                            for kx in range(3):
                                c_lo = max(0, 1 - kx)
                                c_hi = min(W - 1, W + 1 - kx - 1)
                                n_c = c_hi - c_lo + 1
                                nc.tensor.matmul(
                                    M[key][:, :, c_lo:c_lo + n_c],
                                    lhsT=wt_sb[c][:, a, kx, ob * 128:(ob + 1) * 128],
                                    rhs=V[sm][c][a][:, :, c_lo + kx:c_lo + kx + n_c],
                                    start=(c == 0 and kx == 0),
                                    stop=(c == NCH - 1 and kx == 2),
                                )

            def drain(sm, ob):
                # z0 = M0+M1+M2 -> even rows; z1 = M1-M2-M3 -> odd rows.
                # TT ops may read only one PSUM operand: ACT drains M1/M2.
                m = [M[(sm, ob, a)].rearrange("p t w -> p (t w)") for a in range(4)]
                d1 = up.tile([128, 512], F32, name=f"d1_{sm}_{ob}", tag="u", bufs=6)
                d2 = up.tile([128, 512], F32, name=f"d2_{sm}_{ob}", tag="u", bufs=6)
                nc.scalar.copy(d1, m[1])
                nc.scalar.copy(d2, m[2])
                u0 = up.tile([128, 512], F32, name=f"u0_{sm}_{ob}", tag="u", bufs=6)
                z0 = zp.tile([128, 512], BF16, name=f"z0_{sm}_{ob}", tag="z", bufs=4)
                nc.vector.tensor_add(u0, m[0], d1)
                nc.vector.tensor_add(z0, u0, d2)
                u1 = up.tile([128, 512], F32, name=f"u1_{sm}_{ob}", tag="u", bufs=6)
                z1 = zp.tile([128, 512], BF16, name=f"z1_{sm}_{ob}", tag="z", bufs=4)
                nc.vector.tensor_sub(u1, d1, d2)
                nc.vector.tensor_sub(z1, u1, m[3])
                yv = y_sb[sm][ob].rearrange("p (t i) w -> p t i w", i=2)
                dn = den[:, ob, sm:sm + 1]
                nc.scalar.mul(yv[:, :, 0, :], z0.rearrange("p (t w) -> p t w", w=W), dn)
                nc.scalar.mul(yv[:, :, 1, :], z1.rearrange("p (t w) -> p t w", w=W), dn)
                nc.scalar.dma_start(
                    out=y_d[sm, ob],
                    in_=y_sb[sm][ob].rearrange("p a b -> p (a b)"))

            wave(0)
            drain(0, 0)
            drain(0, 1)
            wave(1)
            drain(1, 0)
            drain(1, 1)

    nc.compile()
    return nc


_G = np.array(
    [[1.0, 0.0, 0.0], [0.5, 0.5, 0.5], [0.5, -0.5, 0.5], [0.0, 0.0, 1.0]],
    np.float32)


def prepare_in_maps(x, s, w):
    """Shard + pack full inputs into per-core in_maps (core = g*QGRID + h)."""
    x = np.asarray(x, dtype=np.float32)
    s = np.asarray(s, dtype=np.float32)
    w = np.asarray(w, dtype=np.float32)

    # h-transformed weights: wt1[a,kx,i,o] = sum_p G[a,p] w[o,i,p,kx]
    wt1 = np.einsum("ap,oipk->akio", _G, w, optimize=True)
    # -> [cin, a, kx, cout] -> [NCH,128,4,3,COUT]
    wt_l = np.ascontiguousarray(wt1.transpose(2, 0, 1, 3)).reshape(
        NCH, 128, 4, 3, COUT).astype(BF)
    wsq = np.sum(w * w, axis=(2, 3)).T  # [cin, cout]
    wsq_l = np.ascontiguousarray(
        wsq.reshape(NCH, 128, COUT).transpose(1, 0, 2)).astype(BF)

    x_bf = x.astype(BF).reshape(PGRID, SPC, NCH, 128, H, W)
    xp_all = np.zeros((PGRID, SPC, NCH, 128, HP, HP), BF)
    xp_all[:, :, :, :, 1:H + 1, 1:W + 1] = x_bf
    s_all = s.reshape(PGRID, SPC, NCH, 128).transpose(0, 3, 2, 1)

    in_maps = []
    for g in range(PGRID):
        for h in range(QGRID):
            in_maps.append({
                "xp": np.ascontiguousarray(xp_all[g]),
                "sp": np.ascontiguousarray(s_all[g]),
                "wt": np.ascontiguousarray(wt_l[:, :, :, :, h * OHALF:(h + 1) * OHALF]),
                "wsq": np.ascontiguousarray(wsq_l[:, :, h * OHALF:(h + 1) * OHALF]),
            })
    return in_maps


def assemble_output(results):
    y = np.zeros((B, COUT, H, W), np.float32)
    for g in range(PGRID):
        for h in range(QGRID):
            r = results[g * QGRID + h]["y"].astype(np.float32)
            for sm in range(SPC):
                for ob in range(OB):
                    y[g * SPC + sm,
                      h * OHALF + ob * 128:h * OHALF + (ob + 1) * 128] = (
                        r[sm, ob].reshape(128, H, W))
    return y


def kernel(x, s, w):
    from concourse.bass_utils import run_bass_kernel_spmd

    global _compiled_nc
    if _compiled_nc is None:
        _compiled_nc = _build()
    nc = _compiled_nc

    in_maps = prepare_in_maps(x, s, w)
    res = run_bass_kernel_spmd(nc, in_maps, list(range(B))).results
    return assemble_output(res)


# revision 10
# speedup vs baseline: 1.7780x; 1.0905x over previous
"""Trainium2 Bass kernel for per-sample modulated/demodulated 3x3 conv.

Problem: x (8,512,32,32), s (8,512), w (512,512,3,3) ->
  wm[b,o,i,ky,kx] = w * (s[b,i]+1); demod by rsqrt(sum wm^2 + eps) per (b,o);
  y[b] = conv2d_same(x[b], wm[b]).

1D Winograd F(2,3) along H: y = A^T [ (G_h w) . (B_h^T x') ] with the kx
taps handled directly via shifted-window PSUM accumulation (like a plain
conv), modulation folded into x (x' = x*(1+s)) and demodulation folded
into the output (y *= den[o], den = rsqrt(sum_i (1+s_i)^2 wsq[i,o]+eps)).
The h-transform of the weights (G w along ky -> 4 positions) and wsq are
weight-only preprocessing done host-side; weights ship bf16. PE work is
2/3 of a direct conv; the input transform is 4 contiguous-stride DVE ops
per (cin chunk, sample) (stride-2 only on the middle dim, which DVE runs
at full rate -- inner-stride-2 patterns measured ~2x slower).

Sharding: 4 batch-groups (2 samples each) x 2 cout-halves = 8 cores; each
core loads half the transformed weights (3 MiB bf16) and its 2 samples.

Per-core schedule: two PE waves (sample 0, then sample 1), 8 PSUM banks
per wave (2 cout blocks x 4 h-positions), each bank accumulating 12
matmuls (4 cin chunks x 3 kx shifts) of ~512 bf16 columns. DVE makes
V = B^T x' per chunk (sample 0 first) so wave A starts after one chunk's
4 transform ops. Output transform A^T (z0 = M0+M1+M2 even rows,
z1 = M1-M2-M3 odd rows) splits PSUM reads one-per-op (HW limit): ACT
drains M1/M2 to SBUF, DVE does the adds, ACT applies den and scatters
into even/odd row pairs of the bf16 y tile.
"""

import sys

if "/opt/trn_rl_repo" not in sys.path:
    sys.path.insert(0, "/opt/trn_rl_repo")

import numpy as np
import ml_dtypes

B = 8
CIN = 512
COUT = 512
H = 32
W = 32
NCH = CIN // 128   # cin chunks
PGRID = 4          # batch groups
QGRID = 2          # cout halves
SPC = B // PGRID   # samples per core
OHALF = COUT // QGRID
OB = OHALF // 128  # cout blocks per core
HP = H + 2
EPS = 1e-8
BF = ml_dtypes.bfloat16

_compiled_nc = None


def _build():
    import concourse.tile as tile
    from concourse import bacc, mybir

    F32 = mybir.dt.float32
    BF16 = mybir.dt.bfloat16

    nc = bacc.Bacc("TRN2", target_bir_lowering=False, debug=False, num_devices=B)
    xp_d = nc.dram_tensor("xp", [SPC, NCH, 128, HP, HP], BF16, kind="ExternalInput").ap()
    s1_d = nc.dram_tensor("s1p", [128, NCH, SPC], F32, kind="ExternalInput").ap()
    q_d = nc.dram_tensor("qp", [128, NCH, SPC], BF16, kind="ExternalInput").ap()
    wt_d = nc.dram_tensor("wt", [NCH, 128, 4, 3, OHALF], BF16, kind="ExternalInput").ap()
    wsq_d = nc.dram_tensor("wsq", [128, NCH, OHALF], BF16, kind="ExternalInput").ap()
    y_d = nc.dram_tensor("y", [SPC, OB, 128, H * W], BF16, kind="ExternalOutput").ap()

    with tile.TileContext(nc) as tc:
        with (
            tc.tile_pool(name="wpool", bufs=1) as wpool,
            tc.tile_pool(name="xstage", bufs=4) as xstage,
            tc.tile_pool(name="xmp", bufs=1) as xmp,
            tc.tile_pool(name="vp", bufs=1) as vp,
            tc.tile_pool(name="up", bufs=6) as up,
            tc.tile_pool(name="zp", bufs=4) as zp,
            tc.tile_pool(name="yp", bufs=1) as yp,
            tc.tile_pool(name="misc", bufs=1) as misc,
            tc.tile_pool(name="psum", bufs=8, space="PSUM") as psum,
        ):
            wt_sb = [
                wpool.tile([128, 4, 3, OHALF], BF16, name=f"wt{c}", tag=f"wt{c}")
                for c in range(NCH)
            ]
            xm = [
                [xmp.tile([128, HP, HP], BF16, name=f"xm{sm}_{c}", tag=f"xm{sm}_{c}")
                 for c in range(NCH)]
                for sm in range(SPC)
            ]
            # V[sm][c][a]: h-transformed input, [128, 16 th, 34 w(padded)]
            V = [
                [[vp.tile([128, 16, HP], BF16, name=f"v{sm}_{c}_{a}", tag=f"v{sm}_{c}_{a}")
                  for a in range(4)]
                 for c in range(NCH)]
                for sm in range(SPC)
            ]
            y_sb = [
                [yp.tile([128, H, W], BF16, name=f"y{sm}_{ob}", tag=f"y{sm}_{ob}")
                 for ob in range(OB)]
                for sm in range(SPC)
            ]
            s1 = misc.tile([128, NCH, SPC], F32, name="s1", tag="s1")
            q = misc.tile([128, NCH, SPC], BF16, name="q", tag="q")
            wsq_sb = misc.tile([128, NCH, OHALF], BF16, name="wsq_sb", tag="wsq_sb")
            den_s = misc.tile([128, OB, SPC], F32, name="den_s", tag="den_s")
            den = misc.tile([128, OB, SPC], F32, name="den", tag="den")
            eps_t = misc.tile([128, 1], F32, name="eps_t", tag="eps_t")
            junk = misc.tile([128, 512], BF16, name="junk", tag="junk")
            scr = misc.tile([128, 1], F32, name="scr", tag="scr")

            nc.vector.memset(eps_t, EPS)
            nc.vector.memset(junk, 0.0)
            # prewarm the ACT sqrt table during the DMA shadow so the real
            # demod sqrt doesn't pay the 1.5us ACT_TABLE_LOAD on-path
            nc.scalar.activation(scr, eps_t,
                                 mybir.ActivationFunctionType.Sqrt, bias=eps_t)

            # --- PE warmup while the first DMAs land (HAM clock ramp):
            # many small matmuls keep PE continuously busy cheaply
            warm = psum.tile([128, 512], F32, name="warm", tag="acc")
            for _ in range(20):
                nc.tensor.matmul(warm[:, 0:256], lhsT=junk[:, 0:128],
                                 rhs=junk[:, 0:256], start=True, stop=True)

            # --- DMAs: s1/q/wsq/wt on sync, x on the gpsimd SWDGE queue.
            nc.sync.dma_start(out=s1, in_=s1_d)
            nc.sync.dma_start(out=q, in_=q_d)
            nc.sync.dma_start(out=wsq_sb, in_=wsq_d)
            # c0 weights per h-position so wave A's first matmuls unblock early
            for a in range(4):
                nc.sync.dma_start(out=wt_sb[0][:, a, :, :], in_=wt_d[0, :, a, :, :])
            xs = {}
            for sm in range(SPC):          # sample 0 chunks first
                for c in range(NCH):
                    t = xstage.tile([128, HP, HP], BF16, name=f"xs{sm}_{c}",
                                    tag="xstage", bufs=4)
                    xs[(sm, c)] = t
                    nc.gpsimd.dma_start(out=t, in_=xp_d[sm, c])
            for c in range(1, NCH):
                nc.sync.dma_start(out=wt_sb[c], in_=wt_d[c])

            # --- demod matvec: dsum[ob] = sum_c wsq[c,ob-block]^T (1+s)^2
            dsum = [psum.tile([128, SPC], F32, name=f"dsum{ob}", tag="acc")
                    for ob in range(OB)]
            for ob in range(OB):
                for c in range(NCH):
                    nc.tensor.matmul(
                        dsum[ob],
                        lhsT=wsq_sb[:, c, ob * 128:(ob + 1) * 128],
                        rhs=q[:, c, :],
                        start=(c == 0), stop=(c == NCH - 1),
                    )
            for ob in range(OB):
                nc.scalar.activation(den_s[:, ob, :], dsum[ob],
                                     mybir.ActivationFunctionType.Sqrt, bias=eps_t)

            # --- modulation + input transform, all on DVE (ACT's per-element
            # rate is worse and its table loads would gate the start), sample
            # 0 first so wave A unblocks after one chunk
            for sm in range(SPC):
                for c in range(NCH):
                    nc.vector.tensor_scalar_mul(xm[sm][c], xs[(sm, c)],
                                                s1[:, c, sm:sm + 1])
                    xr = xm[sm][c].rearrange("p (a b) w -> p a b w", b=2)
                    ev0, ev1 = xr[:, 0:16, 0, :], xr[:, 1:17, 0, :]
                    od0, od1 = xr[:, 0:16, 1, :], xr[:, 1:17, 1, :]
                    Vc = V[sm][c]
                    nc.vector.tensor_sub(Vc[0], ev0, ev1)
                    nc.vector.tensor_add(Vc[1], od0, ev1)
                    nc.vector.tensor_sub(Vc[2], ev1, od0)
                    nc.vector.tensor_sub(Vc[3], od0, od1)
                if sm == 0:
                    nc.vector.reciprocal(den, den_s)

            # --- conv waves: per sample, 8 banks = (2 ob x 4 a), each
            # accumulating 12 matmuls (4 c x 3 kx shifted windows). The
            # zero-pad cols of V contribute nothing, so kx=0 skips out col 0
            # and kx=2 skips out col 31 (PSUM has_written covers first hits).
            M = {}

            def wave(sm):
                for c in range(NCH):
                    # kx-outer / bank-inner: consecutive matmuls hit different
                    # PSUM banks (same-bank back-to-back accumulation costs a
                    # turnaround gap on the PE)
                    for kx in range(3):
                        c_lo = max(0, 1 - kx)
                        c_hi = min(W - 1, W + 1 - kx - 1)
                        n_c = c_hi - c_lo + 1
                        for ob in range(OB):
                            for a in range(4):
                                key = (sm, ob, a)
                                if key not in M:
                                    M[key] = psum.tile([128, 16, W], F32,
                                                       name=f"m{sm}_{ob}_{a}",
                                                       tag="acc")
                                nc.tensor.matmul(
                                    M[key][:, :, c_lo:c_lo + n_c],
                                    lhsT=wt_sb[c][:, a, kx, ob * 128:(ob + 1) * 128],
                                    rhs=V[sm][c][a][:, :, c_lo + kx:c_lo + kx + n_c],
                                    start=(c == 0 and kx == 0),
                                    stop=(c == NCH - 1 and kx == 2),
                                )

            def drain(sm, ob):
                # z0 = M0+M1+M2 -> even rows; z1 = M1-M2-M3 -> odd rows.
                # Only the M0+M1 pair needs a staged operand (TT ops may read
                # one PSUM input): ACT drains M1, everything else reads one
                # bank. Split by th-half so the tail chain pipelines and each
                # y half ships as soon as it's scaled.
                dn = den[:, ob, sm:sm + 1]
                yv = y_sb[sm][ob].rearrange("p (t i) w -> p t i w", i=2)
                for hh in range(2):
                    t0 = 8 * hh
                    m = [M[(sm, ob, a)][:, t0:t0 + 8, :] for a in range(4)]
                    d1 = up.tile([128, 8, W], F32, name=f"d1_{sm}_{ob}_{hh}",
                                 tag="u", bufs=6)
                    nc.scalar.copy(d1, m[1])
                    u0 = up.tile([128, 8, W], F32, name=f"u0_{sm}_{ob}_{hh}",
                                 tag="u", bufs=6)
                    z0 = zp.tile([128, 8, W], BF16, name=f"z0_{sm}_{ob}_{hh}",
                                 tag="z", bufs=4)
                    nc.vector.tensor_add(u0, m[0], d1)
                    nc.vector.tensor_add(z0, u0, m[2])
                    u1 = up.tile([128, 8, W], F32, name=f"u1_{sm}_{ob}_{hh}",
                                 tag="u", bufs=6)
                    z1 = zp.tile([128, 8, W], BF16, name=f"z1_{sm}_{ob}_{hh}",
                                 tag="z", bufs=4)
                    nc.vector.tensor_sub(u1, d1, m[2])
                    nc.vector.tensor_sub(z1, u1, m[3])
                    nc.scalar.mul(yv[:, t0:t0 + 8, 0, :], z0, dn)
                    nc.scalar.mul(yv[:, t0:t0 + 8, 1, :], z1, dn)
                    nc.sync.dma_start(
                        out=y_d[sm, ob][:, 512 * hh:512 * (hh + 1)],
                        in_=y_sb[sm][ob].rearrange("p a b -> p (a b)")[
                            :, 512 * hh:512 * (hh + 1)])

            wave(0)
            drain(0, 0)
            drain(0, 1)
            wave(1)
            drain(1, 0)
            drain(1, 1)

    nc.compile()
    return nc


_G = np.array(
    [[1.0, 0.0, 0.0], [0.5, 0.5, 0.5], [0.5, -0.5, 0.5], [0.0, 0.0, 1.0]],
    np.float32)


def prepare_in_maps(x, s, w):
    """Shard + pack full inputs into per-core in_maps (core = g*QGRID + h)."""
    x = np.asarray(x, dtype=np.float32)
    s = np.asarray(s, dtype=np.float32)
    w = np.asarray(w, dtype=np.float32)

    # h-transformed weights: wt1[a,kx,i,o] = sum_p G[a,p] w[o,i,p,kx]
    wt1 = np.einsum("ap,oipk->akio", _G, w, optimize=True)
    # -> [cin, a, kx, cout] -> [NCH,128,4,3,COUT]
    wt_l = np.ascontiguousarray(wt1.transpose(2, 0, 1, 3)).reshape(
        NCH, 128, 4, 3, COUT).astype(BF)
    wsq = np.sum(w * w, axis=(2, 3)).T  # [cin, cout]
    wsq_l = np.ascontiguousarray(
        wsq.reshape(NCH, 128, COUT).transpose(1, 0, 2)).astype(BF)

    x_bf = x.astype(BF).reshape(PGRID, SPC, NCH, 128, H, W)
    xp_all = np.zeros((PGRID, SPC, NCH, 128, HP, HP), BF)
    xp_all[:, :, :, :, 1:H + 1, 1:W + 1] = x_bf
    s1_all = (s + 1.0).reshape(PGRID, SPC, NCH, 128).transpose(0, 3, 2, 1)
    q_all = (s1_all * s1_all).astype(BF)

    in_maps = []
    for g in range(PGRID):
        for h in range(QGRID):
            in_maps.append({
                "xp": np.ascontiguousarray(xp_all[g]),
                "s1p": np.ascontiguousarray(s1_all[g]),
                "qp": np.ascontiguousarray(q_all[g]),
                "wt": np.ascontiguousarray(wt_l[:, :, :, :, h * OHALF:(h + 1) * OHALF]),
                "wsq": np.ascontiguousarray(wsq_l[:, :, h * OHALF:(h + 1) * OHALF]),
            })
    return in_maps


def assemble_output(results):
    y = np.zeros((B, COUT, H, W), np.float32)
    for g in range(PGRID):
        for h in range(QGRID):
            r = results[g * QGRID + h]["y"].astype(np.float32)
            for sm in range(SPC):
                for ob in range(OB):
                    y[g * SPC + sm,
                      h * OHALF + ob * 128:h * OHALF + (ob + 1) * 128] = (
                        r[sm, ob].reshape(128, H, W))
    return y


def kernel(x, s, w):
    from concourse.bass_utils import run_bass_kernel_spmd

    global _compiled_nc
    if _compiled_nc is None:
        _compiled_nc = _build()
    nc = _compiled_nc

    in_maps = prepare_in_maps(x, s, w)
    res = run_bass_kernel_spmd(nc, in_maps, list(range(B))).results
    return assemble_output(res)
